# revision 56
# baseline (speedup 1.0000x reference)
"""Trainium2 Bass kernel for ChannelSpatialSELayer (cSE + sSE squeeze-excite).

    out = max(x * sigmoid(MLP(mean_dhw(x))),          # channel gate (per b, c)
              x * sigmoid(conv_w . x + conv_b))       # spatial gate (per b,d,h,w)

Sharding: pure data parallel over the 64 (batch, depth) slices -> 8 slices
per core.  Cores 0-3 hold batch 0, cores 4-7 hold batch 1.  The only
cross-core dependency is the channel mean, whose per-core partial sums
(128 floats) are AllReduced within each batch's 4-core replica group.

Per core, x stays resident in SBUF between the stats pass and the apply
pass, so HBM traffic is one read + one write of the shard (~63us each at
the ~300 B/ns DMA roofline).  The wall-clock structure is
  loads -> (channel-sum collective + tiny MLP) -> stores
and the middle collective chain (8-40us of latency for 512 bytes, plus an
uncontrollable ~20-90us nrt bootstrap barrier that gates the first
collective) is the only part that cannot overlap the DMA phases.
Design points (each validated against perfetto/NTFF traces):
  * all big matmuls run as fp32r (1 PE cycle/row vs 4 for fp32: PE was
    the original bottleneck at 70% busy); fp32r requires fp32r-typed
    producers and PSUM dst partition base 0,
  * the spatial-gate table sqb is fp16 (sigmoid in (0,1); the fp32r
    broadcast already rounds similarly), freeing SBUF for pipelining,
  * loads run on TWO hwdge queues (SP: pairs 0,3; ACT: pairs 1,2 — the
    ACT issues emitted before any sigmoids reach its FIFO), ending the
    load phase at ~52us; reduces are emitted in data-ARRIVAL order and
    pair 1's run on ACT (in-place fp32r Copy with accum_out) because
    ~39us of DVE reduce work no longer fits inside the load window,
  * the cross-core channel sum is ONE AllGather (measures <= AllReduce;
    single-phase) fired right after the last reduce, summed locally with
    a K=4 ones-matmul on PE; gather/result DMAs are emitted BEFORE the
    pass-2 loop (the SP queue would otherwise head-of-line block them
    behind store issues that themselves wait on the gate -> deadlock),
  * pass-2 prework (4 gate-independent 2048-wide t2 = x*gs super-chunks
    of the LAST loaded pair) is emitted before the MLP so DVE/PE/Pool
    work through the collective window; their stts follow the MLP in
    program order,
  * pass-2 super-chunks: mults stay 1024-wide (PSUM bank limit on the
    gs broadcast) but the fused multiply-max and the store span 2048,
    amortizing DVE per-op overhead (20 stts instead of 36),
  * chunk work splits between DVE (stt + some mults) and Pool+ACT
    (PSUM-copy + mult) — but sparingly: engines share SBUF bandwidth,
    and a Pool-heavy split measurably stretches every DVE op.
"""

import numpy as np

import concourse.bass as bass
import concourse.mybir as mybir
import concourse.tile as tile
from concourse import bacc
from concourse.bass_utils import run_bass_kernel_spmd

B, C, D, H, W = 2, 64, 32, 96, 96
CR = C // 2
S = H * W                 # 9216 spatial elements per (b, d) slice
NCORES = 8
SL = 8                    # (b, d) slices per core
NPAIR = SL // 2           # 4 resident [128, S] slabs per core
NMEAN = float(D * H * W)  # divisor of the channel mean

MCH = 512                 # sq PSUM chunk = 1 bank (leaves 6 banks for pb)
PCH = 1024                # pass-2 chunk = 2 banks
# load chunks (col offset, cols) per pair: pairs 0-2 use two big DMAs for
# better engine utilization; pair 3 ends with two half-chunks so the final
# reduce on the AllReduce-2 critical path is short.
LOADS_BIG = [(0, 4608), (4608, 4608)]
LOADS_TAIL = [(0, 2304), (2304, 2304), (4608, 2304), (6912, 1152), (8064, 1152)]
# per-pair load plans: arrival order on the fabric is p0, p3 (SP queue),
# p1, p2 (ACT queue) — the globally LAST chunks (pair 2's tail) are small
# so the final reduces on the AllReduce critical path are short.
LOADS_BY_PAIR = [LOADS_BIG, LOADS_BIG, LOADS_TAIL, LOADS_TAIL]
STATS_BASE = [0, 2, 9, 4]
NSTATS = 14
GROUPS = [[0, 1, 2, 3], [4, 5, 6, 7]]  # batch replica groups

F32 = mybir.dt.float32
F16 = mybir.dt.float16
F32R = mybir.dt.float32r  # PE fast path: 1 cycle/row (vs 4 for fp32) at >=256 cols
AX = mybir.AxisListType
AL = mybir.AluOpType
AF = mybir.ActivationFunctionType


def _r(ap):
    return ap.bitcast(F32R)


# pass-2 apply: per chunk, t2 = x*gs then out = (x*gc) max t2 (stt on DVE).
# Pool can't touch PSUM and only supports mult/ts/copy, so chunks split in
# two plans:
#   A: DVE mult reads gs straight from PSUM, DVE stt       (DVE ~2.5us)
#   B: ACT copies PSUM->SBUF, Pool mults x*gs, DVE stt     (ACT 1.1 Pool 2.8 DVE 1.3)
# The 36 stts (~46us) are DVE-bound and gate-dependent; everything else is
# spread so post-AllReduce DVE ~53us and Pool ~53us sit at the ~63us DMA
# store floor.  Pair 3 (the prework pair, whose mults run inside the
# AllReduce window where DVE idles) is all-DVE; pairs 0-2 lean Pool.
def _plan_a(jp, pc):
    if jp == 3:
        return True
    return pc in (0, 2, 5, 8)


def _build(fc1_w, fc1_b, fc2_w, fc2_b, conv_w, conv_b):
    # Bacc (not raw Bass): its compile() pipeline splits multi-sem waits
    # into event semaphores — TRN2 allows at most 1 wait per instruction.
    nc = bacc.Bacc(
        "TRN2",
        target_bir_lowering=False,
        debug=False,
        num_devices=NCORES,
    )
    # [pair, partition, spatial]: the host pre-arranges shards so every DMA's
    # outer dim is the full 128 partitions — the SDMA engine fan-out follows
    # the outer AP dim in groups of 8, so this engages all 16 engines.
    xin = nc.dram_tensor("xin", [NPAIR, 128, S], F32, kind="ExternalInput")
    yout = nc.dram_tensor("yout", [NPAIR, 128, S], F32, kind="ExternalOutput")

    # Host-prepared constants (identical on every core, embedded in the NEFF).
    # w1fold folds the 1/NMEAN of the mean into fc1 and sums the two
    # 64-partition halves (both hold the same batch) in the K=128 contraction.
    w1fold = (np.vstack([fc1_w.T, fc1_w.T]) / NMEAN).astype(np.float32)  # [128,CR]
    w2t = np.ascontiguousarray(fc2_w.T).astype(np.float32)               # [CR,C]
    wsel = np.zeros((128, 2), np.float32)  # sq = wsel.T @ x per slice pair
    wsel[:C, 0] = conv_w
    wsel[C:, 1] = conv_w
    # broadcast-selector: pair jp's two gs rows live at partition base
    # 32*jp (the only legal SBUF engine bases are 0/32/64/96).  lhsT
    # [2, 128] at that base sends row 0 to partitions 0-63 and row 1 to
    # partitions 64-127 of the PSUM output.  fp16: the values are 0/1.
    bselbig = np.zeros((98, 128), np.float16)
    for jp in range(NPAIR):
        bselbig[32 * jp, :C] = 1.0
        bselbig[32 * jp + 1, C:] = 1.0
    b1 = fc1_b.reshape(CR, 1).astype(np.float32)
    b2 = fc2_b.reshape(C, 1).astype(np.float32)
    cb = float(np.asarray(conv_b).reshape(-1)[0])

    w1_d = nc.inline_tensor(w1fold, "w1fold")
    w2_d = nc.inline_tensor(w2t, "w2t")
    wsel_d = nc.inline_tensor(wsel, "wsel")
    bsel_d = nc.inline_tensor(bselbig, "bselbig")
    b1_d = nc.inline_tensor(b1, "b1")
    b2_d = nc.inline_tensor(b2, "b2")

    with tile.TileContext(nc) as tc:
        with (
            tc.tile_pool(name="consts", bufs=1) as consts,
            tc.tile_pool(name="xpool", bufs=1) as xpool,
            tc.tile_pool(name="sqpool", bufs=1) as sqpool,
            tc.tile_pool(name="stp", bufs=1) as stp,
            tc.tile_pool(name="dram", bufs=1, space="DRAM") as dram,
        ):
            xres = xpool.tile([128, NPAIR * S], F32)   # 144 KB/partition
            # spatial gates (fp16): pair jp's rows at partition base 32*jp
            sqb = sqpool.tile([98, S], F16)
            stats = stp.tile([128, NSTATS], F32)

            with (
                tc.tile_pool(name="psq", bufs=2, space="PSUM") as psq,
                tc.tile_pool(name="pb", bufs=3, space="PSUM") as pb,
                tc.tile_pool(name="t2p", bufs=4) as t2p,
                tc.tile_pool(name="t1k", bufs=1) as t1k,
                tc.tile_pool(name="gsp", bufs=3) as gsp,
            ):
                # -------- first pair's loads go out before the consts -------
                def _load_issue(jp, lc, eng=None):
                    c0, n = LOADS_BY_PAIR[jp][lc]
                    dst = xres[:, jp * S + c0 : jp * S + c0 + n]
                    (eng or nc.sync).dma_start(
                        out=_r(dst),
                        in_=_r(xin[jp, :, c0 : c0 + n]),
                    )

                def _load_reduce(jp, lc):
                    c0, n = LOADS_BY_PAIR[jp][lc]
                    dst = xres[:, jp * S + c0 : jp * S + c0 + n]
                    sl = STATS_BASE[jp] + lc
                    if jp == 1:
                        # pair 1's reduces run on ACT (in-place Copy with
                        # accum_out — Pool cannot free-axis-reduce): ~39us
                        # of DVE reduce work no longer fits inside the 52us
                        # dual-queue load window and delays the collective.
                        nc.scalar.activation(
                            out=_r(dst), in_=_r(dst), func=AF.Copy,
                            accum_out=stats[:, sl : sl + 1],
                        )
                    else:
                        nc.vector.reduce_sum(
                            out=stats[:, sl : sl + 1], in_=dst, axis=AX.X
                        )

                def _nloads(jp):
                    return len(LOADS_BY_PAIR[jp])

                for lc in range(_nloads(0)):
                    _load_issue(0, lc)
                    _load_reduce(0, lc)

                # fp32r consumers require fp32r producers (BIR verifier), so
                # DMAs feeding PE matmul operands are typed fp32r end-to-end.
                wsel_sb = consts.tile([128, 2], F32)
                nc.sync.dma_start(out=_r(wsel_sb), in_=_r(wsel_d[:, :]))
                bsel_sb = consts.tile([98, 128], F16)
                nc.sync.dma_start(out=bsel_sb, in_=bsel_d[:, :])
                w1_sb = consts.tile([128, CR], F32)
                nc.sync.dma_start(out=w1_sb, in_=w1_d[:, :])
                w2_sb = consts.tile([CR, C], F32)
                nc.sync.dma_start(out=w2_sb, in_=w2_d[:, :])
                b1_sb = consts.tile([CR, 1], F32)
                nc.sync.dma_start(out=b1_sb, in_=b1_d[:, :])
                b2_sb = consts.tile([C, 1], F32)
                nc.sync.dma_start(out=b2_sb, in_=b2_d[:, :])
                cbB = consts.tile([98, 1], F32)
                nc.vector.memset(cbB, cb)
                ones4 = consts.tile([4, 1], F32)
                nc.vector.memset(ones4, 1.0)

                # ---------- pass 1: loads, channel sums, sq logits ----------
                def _sq(jp):
                    r0 = 32 * jp
                    for mc in range(S // MCH):
                        # fp32r matmuls must write PSUM at partition base 0
                        # (s3d3_mm_valid_dst_partition); the sigmoid shifts
                        # the rows to sqb's per-pair partition base.
                        ps = psq.tile([128, MCH], F32, tag="ps")
                        for i in range(MCH // 512):
                            o = mc * MCH + i * 512
                            nc.tensor.matmul(
                                ps[0:2, i * 512 : (i + 1) * 512],
                                lhsT=_r(wsel_sb),
                                rhs=_r(xres[:, jp * S + o : jp * S + o + 512]),
                                start=True,
                                stop=True,
                                tile_position=(0, 0),
                            )
                        off = mc * MCH
                        nc.scalar.activation(
                            out=sqb[r0 : r0 + 2, off : off + MCH],
                            in_=ps[0:2, :],
                            func=AF.Sigmoid,
                            bias=cbB[r0 : r0 + 2, :],
                            scale=1.0,
                        )

                # ------- channel-sum AllReduce within the batch group -------
                # ONE collective, fired right after the last reduce: with
                # dual-queue loads the load phase ends ~52us, so an early
                # split collective can no longer hide under it — the first
                # op on the CC stream costs ~26us regardless of payload.
                # Emitted BEFORE pass 2: the s_sb DMA must sit ahead of the
                # store issues in the SP FIFO (stores wait on the gate, the
                # gate waits on s_sb -> circular deadlock otherwise), and
                # the Pool trigger ahead of the Pool mults.
                # pairs 1/2 issue from the ACT hwdge queue (second DMA
                # queue, fewer inter-transfer gaps) — emitted BEFORE any
                # sigmoids so the issues sit at the front of the ACT FIFO.
                # Pair 3 issues early on SP.  Reduces are emitted in data
                # ARRIVAL order (p0, p3, p1, p2) so the DVE FIFO never
                # blocks on a late pair while ready data waits behind it.
                for jp in (1, 2):
                    for lc in range(_nloads(jp)):
                        _load_issue(jp, lc, nc.scalar)
                for lc in range(_nloads(3)):
                    _load_issue(3, lc)
                for jp in (3, 2):
                    for lc in range(_nloads(jp)):
                        _load_reduce(jp, lc)
                _sq(0)
                for lc in range(_nloads(1)):
                    _load_reduce(1, lc)
                _sq(3)
                _sq(1)
                _sq(2)

                ssum = stp.tile([128, 1], F32)
                nc.vector.reduce_sum(out=ssum, in_=stats[:, 0:NSTATS], axis=AX.X)
                b_in = dram.tile([128, 1], F32)
                b_g = dram.tile([4, 128], F32)
                nc.sync.dma_start(out=b_in, in_=ssum)
                # AllGather measures consistently faster than AllReduce for
                # this 512B payload (single phase); the 4-rank sum happens
                # locally on PE (K=4 ones-matmul) after the gather.
                nc.gpsimd.collective_compute(
                    "AllGather",
                    AL.bypass,
                    replica_groups=GROUPS,
                    ins=[b_in.opt()],
                    outs=[b_g.opt()],
                )
                g_sb = stp.tile([4, 128], F32)
                nc.sync.dma_start(out=g_sb, in_=b_g)
                s_sb = stp.tile([128, 1], F32)

                g2_sb = stp.tile([128, 1], F32)

                # ------- pass 2: apply both gates, stream out ---------------
                # t2 = x*gs is gate-independent, so the first PREWORK chunks'
                # broadcasts/copies/mults are emitted BEFORE the MLP: the
                # PE/ACT/Pool/DVE FIFOs then have runnable work while the
                # AllReduce is in flight.  Their stts (which read g2_sb and so
                # must follow the MLP writes in program order) come after.
                # Super-chunks: mults stay 1024-wide (PSUM bank limit on
                # the broadcast) but the stt and the store span 2048 where
                # possible — fewer DVE ops amortize the ~215ns fixed cost
                # and stores go out as 1MB transfers.  9216 = 4*2048 + 1024.
                def _front(jp, o, w):
                    r0 = 32 * jp
                    pool = t2p if w == 2048 else t1k
                    t2 = pool.tile([128, w], F32)
                    for h in range(w // PCH):
                        oh = o + h * PCH
                        xc = xres[:, jp * S + oh : jp * S + oh + PCH]
                        g_ps = pb.tile([128, PCH], F32)
                        for i in range(PCH // 512):
                            nc.tensor.matmul(
                                g_ps[:, i * 512 : (i + 1) * 512],
                                lhsT=bsel_sb[r0 : r0 + 2, :],
                                rhs=sqb[r0 : r0 + 2, oh + i * 512 : oh + (i + 1) * 512],
                                start=True,
                                stop=True,
                                tile_position=(r0, 0),
                            )
                        t2h = t2[:, h * PCH : (h + 1) * PCH]
                        if _plan_a(jp, oh // PCH):
                            nc.vector.tensor_mul(out=t2h, in0=xc, in1=g_ps)
                        else:
                            gs_sb = gsp.tile([128, PCH], F16)
                            nc.scalar.activation(out=gs_sb, in_=g_ps, func=AF.Copy)
                            nc.gpsimd.tensor_mul(out=t2h, in0=xc, in1=gs_sb)
                    return t2

                def _back(jp, o, w, t2):
                    # (stores must read SBUF — DMA cannot source from PSUM,
                    # so the stt result stays in t2 in place)
                    xc = xres[:, jp * S + o : jp * S + o + w]
                    nc.vector.scalar_tensor_tensor(
                        out=t2,
                        in0=xc,
                        scalar=g2_sb,
                        in1=t2,
                        op0=AL.mult,
                        op1=AL.max,
                    )
                    nc.sync.dma_start(out=yout[jp, :, o : o + w], in_=t2)

                # Prework chunks come from the LAST pair: their sqb rows only
                # become ready as the loads finish, so the prework cannot
                # contend with the load phase for SBUF bandwidth — it lands
                # exactly in the AllReduce window.
                SUP = [(o * 2048, 2048) for o in range(4)] + [(8192, 1024)]
                chunks = [(3, o, w) for o, w in SUP] + [
                    (jp, o, w) for jp in range(NPAIR - 1) for o, w in SUP
                ]
                # 4 supers (== t2p bufs) + pair 3's 1024 tail (sole t1k
                # user before its own stt frees the buffer — deadlock-safe)
                PREWORK = 5
                pre_t2 = [_front(jp, o, w) for jp, o, w in chunks[:PREWORK]]

                # ------- tiny cSE MLP -> per-partition channel gate ---------
                # gc [64] is duplicated to 128 partitions by running the
                # sigmoid twice with a partition-shifted output.
                mts = psq.tile([128, MCH], F32, tag="ps")
                nc.tensor.matmul(
                    mts[:, 0:1], lhsT=g_sb, rhs=ones4, start=True, stop=True
                )
                nc.scalar.activation(out=s_sb, in_=mts[:, 0:1], func=AF.Copy)
                mt1 = psq.tile([128, MCH], F32, tag="ps")
                nc.tensor.matmul(
                    mt1[:CR, 0:1], lhsT=w1_sb, rhs=s_sb, start=True, stop=True
                )
                h_sb = stp.tile([CR, 1], F32)
                nc.scalar.activation(
                    out=h_sb, in_=mt1[:CR, 0:1], func=AF.Relu, bias=b1_sb, scale=1.0
                )
                mt2 = psq.tile([128, MCH], F32, tag="ps")
                nc.tensor.matmul(
                    mt2[:C, 0:1], lhsT=w2_sb, rhs=h_sb, start=True, stop=True
                )
                nc.scalar.activation(
                    out=g2_sb[0:C, :], in_=mt2[:C, 0:1], func=AF.Sigmoid,
                    bias=b2_sb, scale=1.0,
                )
                nc.scalar.activation(
                    out=g2_sb[C:128, :], in_=mt2[:C, 0:1], func=AF.Sigmoid,
                    bias=b2_sb, scale=1.0,
                )

                for (jp, o, w), t2 in zip(chunks[:PREWORK], pre_t2):
                    _back(jp, o, w, t2)
                for jp, o, w in chunks[PREWORK:]:
                    t2 = _front(jp, o, w)
                    _back(jp, o, w, t2)
    # run Bacc's compile pipeline (register allocation, wait splitting);
    # the bass2jax/PJRT runner does not finalize on its own.
    nc.finalize()
    return nc


def _shard(x):
    # core k shard: xin[jp, 64*t + c, s] = x[b, c, d0 + 2*jp + t, s]
    in_maps = []
    for k in range(NCORES):
        b, d0 = k // 4, SL * (k % 4)
        v = x[b, :, d0 : d0 + SL].reshape(C, NPAIR, 2, S)
        shard = np.ascontiguousarray(v.transpose(1, 2, 0, 3).reshape(NPAIR, 128, S))
        in_maps.append({"xin": shard})
    return in_maps


def _unshard(results):
    out = np.empty((B, C, D, H, W), np.float32)
    for k in range(NCORES):
        b, d0 = k // 4, SL * (k % 4)
        y = results[k]["yout"].reshape(NPAIR, 2, C, S)
        out[b, :, d0 : d0 + SL] = y.transpose(2, 0, 1, 3).reshape(C, SL, H, W)
    return out


def _run(inputs, trace=False):
    x = np.ascontiguousarray(np.asarray(inputs["input_tensor"], dtype=np.float32))
    ws = [
        np.asarray(inputs[k], dtype=np.float32)
        for k in ("fc1_w", "fc1_b", "fc2_w", "fc2_b", "conv_w", "conv_b")
    ]
    nc = _build(*ws)
    res = run_bass_kernel_spmd(nc, _shard(x), list(range(NCORES)), trace=trace)
    return _unshard(res.results), res


def kernel(**inputs):
    out, _ = _run(inputs, trace=False)
    return out
